# revision 1
# baseline (speedup 1.0000x reference)
"""Euler-characteristic-curve kernel for Trainium2 (Bass/Tile).

Algorithm
---------
Per (batch, channel) group, reference computes
    cover(t_k) = #{n : birth_n < t_k <= death_n},  t_k = k/255 (f32), k=0..255
and the output is cover_pd0 - cover_pd1.

Identity: [b < t][d >= t] = [b < t] - [max(b,d) < t], so
    cover(t_k) = Cb(t_k) - Cm(t_k),   Cv(t_k) = #{n : v_n < t_k}.
Cv is a cumulative histogram: with q(v) = the exact index s.t.
t_q <= v < t_{q+1}, we have  Cv(t_k) = #{n : q(v_n) < k}.

On device, per point: q = floor(v*255) corrected by exact comparisons
against t_c = f32(c) * f32(1/255) (bitwise identical to the reference's
jnp.linspace grid -- verified).  q is split into nibbles qh = q >> 4,
ql = q & 15.  The 16x16 joint histogram H[qh, ql] is computed as a
matmul of one-hot(qh) x one-hot(ql) tiles contracted over points
(128 points/pass, 4 groups + both value-arrays packed per pass).
The 256-bin cumulative count is reassembled as
    C(16K+L) = sum_{h<K} rowsum(H[h,:]) + prefix(H[K,:])[L-1]
via a tiny strict-triangular matmul + per-row prefix scans.

Sharding: data-parallel over batch, 4 batches per core x 8 cores.
"""

import os
import sys

for _p in ("/opt/trn_rl_repo", os.path.expanduser("~/.axon_site/_ro/trn_rl_repo")):
    if os.path.isdir(_p) and _p not in sys.path:
        sys.path.insert(0, _p)

import numpy as np
import ml_dtypes

import concourse.bass as bass
import concourse.bacc as bacc
import concourse.mybir as mybir
from concourse.tile import TileContext
from concourse.bass_utils import run_bass_kernel_spmd

NCORES = 8
B, C, N = 32, 3, 8192
TT = 256                      # thresholds
NG = (B // NCORES) * C        # 12 groups (b,c pairs) per diagram per core
NI = N // 128                 # 64 point-slices of 128 per group
GSET = 4                      # groups packed per matmul pass
NSET = NG // GSET             # 3 sets per diagram
R = float(np.float32(1.0) / np.float32(255.0))

F32 = mybir.dt.float32
BF16 = mybir.dt.bfloat16
OP = mybir.AluOpType


def build_nc():
    nc = bacc.Bacc("TRN2", target_bir_lowering=False, debug=False)
    pds = [
        nc.dram_tensor(f"pd{d}", [NG, N, 2], F32, kind="ExternalInput")
        for d in range(2)
    ]
    iota_d = nc.dram_tensor("iotaw", [128, 16 * 8], BF16, kind="ExternalInput")
    tri_d = nc.dram_tensor("tri", [16, 16], F32, kind="ExternalInput")
    sel_d = nc.dram_tensor("sel", [128, 256], F32, kind="ExternalInput")
    out_d = nc.dram_tensor("out", [NG, TT], F32, kind="ExternalOutput")

    with TileContext(nc) as tc:
        with (
            tc.tile_pool(name="consts", bufs=1) as cpool,
            tc.tile_pool(name="src", bufs=3) as spool,
            tc.tile_pool(name="tmp", bufs=2) as tpool,
            tc.tile_pool(name="idx", bufs=3) as ipool,
            tc.tile_pool(name="oh", bufs=4) as ohpool,
            tc.tile_pool(name="psum", bufs=4, space="PSUM") as ppool,
            tc.tile_pool(name="psc", bufs=2, space="PSUM") as pcpool,
            tc.tile_pool(name="post", bufs=2) as qpool,
        ):
            iotaw = cpool.tile([128, 16 * 8], BF16)
            tri = cpool.tile([16, 16], F32)
            sel = cpool.tile([128, 256], F32)
            warm = cpool.tile([128, 1], F32)

            # net histograms (Hb-Hm)_pd0 - (Hb-Hm)_pd1 for the 12 group
            # pairs, accumulated by +/-1 selection matmuls (the pd1 pass
            # uses the negated sel block, folding the diagram subtraction
            # into PSUM accumulation)
            pnet = pcpool.tile([16, NG * 16], F32, tag="pnet")

            NCH = 2          # one-hot/matmul chunks per set
            ICH = NI // NCH  # i-slices per chunk
            W = GSET * 128
            pending = []

            def _extract_pair(ps0, ps1, sd, eng=None):
                # aligned PSUM->SBUF copies, then +/-1 selection matmuls.
                # Rows/cols of each histogram square are interleaved
                # (8K + j, j = 2g+v): sel stationary picks rows 8K+j, the
                # moving operand strides the columns.  pd1 uses the negated
                # sel block; each pair-column's 4 matmuls run consecutively
                # so only one PSUM accumulation group is open per region.
                ssbs = []
                for ps in (ps0, ps1):
                    ssb = ohpool.tile([128, 128], F32, tag="ssb")
                    if eng is None:
                        nc.scalar.copy(ssb[:, :], ps[:, :])
                    else:
                        eng.tensor_copy(ssb[:, :], ps[:, :])
                    ssbs.append(ssb[:, :].rearrange("p (L j) -> p L j", j=8))
                for gl in range(GSET):
                    gp = sd * GSET + gl
                    for d in range(2):
                        for v in range(2):
                            j = 2 * gl + v
                            c0 = 128 * d + 16 * j
                            nc.tensor.matmul(
                                pnet[:, 16 * gp : 16 * gp + 16],
                                sel[:, c0 : c0 + 16],
                                ssbs[d][:, :, j],
                                start=(d == 0 and v == 0),
                                stop=(d == 1 and v == 1),
                            )

            z16 = qpool.tile([16, 16], F32, tag="z16")
            nc.vector.memset(z16[:, :], 0.0)

            def _post_pair(sd):
                # finish groups [4sd, 4sd+4): net hist -> cumulative counts;
                # the scans read the net histogram straight out of PSUM
                g0 = GSET * sd
                pnet_v = pnet[:, :].rearrange("p (g e) -> p g e", e=16)
                scn = qpool.tile([16, GSET, 16], F32, tag="scn")
                for gl in range(GSET):
                    nc.vector.tensor_tensor_scan(
                        scn[:, gl, :], pnet_v[:, g0 + gl, :], z16[:, :], 0.0,
                        OP.add, OP.add,
                    )
                rs = qpool.tile([16, GSET], F32, tag="rs")
                nc.gpsimd.tensor_copy(rs[:, :], scn[:, :, 15])
                ccp = pcpool.tile([16, GSET], F32, tag="ccp")
                nc.tensor.matmul(
                    ccp[:, :], tri[:, :], rs[:, :], start=True, stop=True
                )
                ccs = qpool.tile([16, GSET], F32, tag="ccs")
                nc.scalar.copy(ccs[:, :], ccp[:, :])
                fin = qpool.tile([16, GSET, 16], F32, tag="fin")
                for gl in range(GSET):
                    nc.vector.tensor_scalar(
                        fin[:, gl, 1:16], scn[:, gl, 0:15],
                        ccs[:, gl : gl + 1], None, OP.add,
                    )
                    nc.gpsimd.tensor_copy(fin[:, gl, 0:1], ccs[:, gl : gl + 1])
                nc.sync.dma_start(
                    out_d.ap()[g0 : g0 + GSET, :].rearrange(
                        "g (K L) -> K g L", K=16
                    ),
                    fin[:, :, :],
                )

            hold = {}

            def _finish(item, eng=None):
                ps, d, sd = item
                hold[(sd, d)] = ps
                if (sd, 0) in hold and (sd, 1) in hold:
                    _extract_pair(hold.pop((sd, 0)), hold.pop((sd, 1)), sd, eng)
                    _post_pair(sd)

            for sd in range(NSET):
                    # both diagrams' set sd share one wide prep chain
                    # (halves the per-op fixed overheads)
                    src = spool.tile([128, 2, GSET, 128], F32, tag="src")
                    for d in range(2):
                        nc.sync.dma_start(
                            src[:, d, :, :],
                            pds[d]
                            .ap()[GSET * sd : GSET * (sd + 1), :, :]
                            .rearrange("g (p x) two -> p g (x two)", p=128),
                        )
                    if sd == 0:
                        # consts load behind the first data tiles; a dummy ACT
                        # op preloads the Copy table during the DMA wait
                        nc.sync.dma_start(iotaw[:, :], iota_d.ap())
                        nc.sync.dma_start(tri[:, :], tri_d.ap())
                        nc.sync.dma_start(sel[:, :], sel_d.ap())
                        nc.vector.memset(warm[:, :], 0.0)
                        nc.scalar.mul(warm[:, :], warm[:, :], 2.0)

                    flat = src[:, :, :, :].rearrange("p d g x -> p (d g x)")
                    pairs = src[:, :, :, :].rearrange(
                        "p d g (i two) -> p (d g i) two", two=2
                    )
                    bsl = pairs[:, :, 0:1]
                    dsl = pairs[:, :, 1:2]

                    W2 = 2 * W
                    tmb = tpool.tile([128, W2], F32, tag="tmb")
                    cf = tpool.tile([128, W2], F32, tag="cf")
                    tlo = tpool.tile([128, W2], F32, tag="tlo")
                    lt = tpool.tile([128, W2], F32, tag="lt")
                    qi = tpool.tile([128, W2], mybir.dt.int16, tag="qi")
                    # [p, i, g, v] so one-hot APs merge (g,v); packed last dim
                    qh = ipool.tile([128, 64, 2 * GSET, 2], BF16, tag="qh")
                    ql = ipool.tile([128, 64, 2 * GSET, 2], BF16, tag="ql")

                    qhi = tpool.tile([128, W2], mybir.dt.int16, tag="qhi")
                    qli = tpool.tile([128, W2], mybir.dt.int16, tag="qli")

                    def _prep(g0, ng, dve=False):
                        s = slice(128 * g0, 128 * (g0 + ng))
                        sp = slice(64 * g0, 64 * (g0 + ng))
                        # deaths <- max(birth, death), in the death slot
                        nc.vector.tensor_tensor(
                            dsl[:, sp, :], bsl[:, sp, :], dsl[:, sp, :], OP.max
                        )
                        # c = round(v*255) via fused v*255 + 2^23 (any
                        # rounding order keeps |c - v*255| <= 0.5 + 5e-5,
                        # enough for the one-comparison correction proof)
                        if dve:
                            nc.vector.tensor_scalar(
                                tmb[:, s], flat[:, s], 255.0, 8388608.0,
                                OP.mult, OP.add,
                            )
                            nc.vector.tensor_scalar(
                                cf[:, s], tmb[:, s], 8388608.0, None,
                                OP.subtract,
                            )
                        else:
                            nc.scalar.activation(
                                tmb[:, s], flat[:, s],
                                mybir.ActivationFunctionType.Copy,
                                bias=8388608.0, scale=255.0,
                            )
                            nc.scalar.activation(
                                cf[:, s], tmb[:, s],
                                mybir.ActivationFunctionType.Copy,
                                bias=-8388608.0,
                            )
                        # exact grid value t_c (== reference linspace).
                        # With c = ROUND(fl(v*255)) the true index is c or
                        # c-1 only: q >= c+1 would need v >= t_{c+1}, i.e.
                        # v*255 >= c+1-2e-5, making round() >= c+1; and
                        # q <= c-2 would make round() <= c-1.  So a single
                        # comparison corrects exactly: q = c - [v < t_c].
                        nc.scalar.mul(tlo[:, s], cf[:, s], float(R))
                        nc.vector.tensor_tensor(
                            lt[:, s], flat[:, s], tlo[:, s], OP.is_lt
                        )
                        # q = cf - lt, written straight to int16 (exact);
                        # nibble split: qh = q >> 4, ql = q & 15 (bit-ops
                        # cannot cast; convert+transpose happens in copies)
                        nc.vector.tensor_tensor(
                            qi[:, s], cf[:, s], lt[:, s], OP.subtract
                        )
                        nc.vector.tensor_scalar(
                            qhi[:, s], qi[:, s], 4, None, OP.logical_shift_right
                        )
                        nc.vector.tensor_scalar(
                            qli[:, s], qi[:, s], 15, None, OP.bitwise_and
                        )
                        gs = slice(g0, g0 + ng)
                        qh_w = qh[:, :, gs, :].rearrange("p i g v -> p g i v")
                        ql_w = ql[:, :, gs, :].rearrange("p i g v -> p g i v")
                        qhi_v = qhi[:, s].rearrange(
                            "p (g i v) -> p g i v", g=ng, v=2
                        )
                        qli_v = qli[:, s].rearrange(
                            "p (g i v) -> p g i v", g=ng, v=2
                        )
                        nc.gpsimd.tensor_copy(qh_w, qhi_v)
                        nc.gpsimd.tensor_copy(ql_w, qli_v)

                    # one-hot layout (i, e, gv): every operand's last AP dim
                    # is packed 2-byte -> DVE 2x mode; chunked for pipelining
                    def _ohmm(d, nch=NCH):
                      ich = NI // nch
                      ps = ppool.tile([128, 128], F32, tag="ps")
                      for ch in range(nch):
                        At = ohpool.tile([128, ich, 16, GSET * 2], BF16, tag="A")
                        Bt = ohpool.tile([128, ich, 16, GSET * 2], BF16, tag="B")

                        def _vals(t):
                            ap = t[
                                :, ich * ch : ich * (ch + 1),
                                GSET * d : GSET * (d + 1), :,
                            ].rearrange("p i g v -> p i (g v)")
                            # [p, i, e(bcast), gv]
                            return bass.AP(
                                ap.tensor,
                                ap.offset,
                                [ap.ap[0], ap.ap[1], [0, 16], ap.ap[2]],
                            )

                        io_b = bass.AP(
                            iotaw[:, :].tensor,
                            iotaw[:, :].offset,
                            [iotaw[:, :].ap[0], [0, ich], [8, 16], [1, 8]],
                        )
                        nc.vector.tensor_tensor(
                            At[:, :, :, :], _vals(qh), io_b, OP.is_equal
                        )
                        nc.vector.tensor_tensor(
                            Bt[:, :, :, :], _vals(ql), io_b, OP.is_equal
                        )
                        a_m = At[:, :, :, :].rearrange("p i e gv -> p i (e gv)")
                        b_m = Bt[:, :, :, :].rearrange("p i e gv -> p i (e gv)")
                        for il in range(ich):
                            nc.tensor.matmul(
                                ps[:, :],
                                a_m[:, il, :],
                                b_m[:, il, :],
                                start=(ch == 0 and il == 0),
                                stop=(ch == nch - 1 and il == ich - 1),
                            )

                      # extraction is deferred so the in-order engine
                      # streams never stall on PE matmuls; post-processing
                      # runs per set-pair once both diagrams are extracted
                      pending.append((ps, d, sd))
                      if len(pending) > 2:
                        _finish(pending.pop(0))

                    if sd == 0:
                        # first pair: interleave halves so compute starts
                        # right after the first diagram's DMA lands
                        _prep(0, GSET, dve=True)
                        _ohmm(0)
                        _prep(GSET, GSET)
                        _ohmm(1)
                    else:
                        _prep(0, 2 * GSET)
                        _ohmm(0)
                        _ohmm(1, nch=4 if sd == NSET - 1 else NCH)

            while pending:
                # tail flush: DVE is idle here while ACT would serialize
                _finish(pending.pop(0), eng=nc.vector)
    nc.compile()
    return nc


_NC = None


def _get_nc():
    global _NC
    if _NC is None:
        _NC = build_nc()
    return _NC


def make_in_maps(pd0, pd1):
    pd0 = np.ascontiguousarray(np.asarray(pd0, dtype=np.float32))
    pd1 = np.ascontiguousarray(np.asarray(pd1, dtype=np.float32))
    # iotaw[p, 8e + j] = e  (bin value repeated across the 8 (g,v) slots)
    iotaw = np.tile(
        np.repeat(np.arange(16, dtype=np.float32), 8), (128, 1)
    ).astype(ml_dtypes.bfloat16)
    tri = (np.arange(16)[:, None] < np.arange(16)[None, :]).astype(np.float32)
    # sel[8K + j, 16j + K] = +1 for j even (births), -1 for j odd
    # (max-vals); cols [128:256] are negated for the pd1 accumulation
    csel = np.zeros((128, 256), dtype=np.float32)
    for j in range(8):
        for kk in range(16):
            s = 1.0 if j % 2 == 0 else -1.0
            csel[8 * kk + j, 16 * j + kk] = s
            csel[8 * kk + j, 128 + 16 * j + kk] = -s
    bs = B // NCORES
    in_maps = []
    for c in range(NCORES):
        in_maps.append(
            {
                "pd0": np.ascontiguousarray(
                    pd0[bs * c : bs * (c + 1)].reshape(NG, N, 2)
                ),
                "pd1": np.ascontiguousarray(
                    pd1[bs * c : bs * (c + 1)].reshape(NG, N, 2)
                ),
                "iotaw": iotaw,
                "tri": tri,
                "sel": csel,
            }
        )
    return in_maps


def kernel(pd0, pd1, trace=False):
    nc = _get_nc()
    in_maps = make_in_maps(pd0, pd1)
    res = run_bass_kernel_spmd(nc, in_maps, list(range(NCORES)), trace=trace)
    bs = B // NCORES
    out = np.concatenate(
        [res.results[c]["out"].reshape(bs, C, TT) for c in range(NCORES)], axis=0
    )
    if trace:
        return out.astype(np.float32), res
    return out.astype(np.float32)



# revision 22
# speedup vs baseline: 1.2120x; 1.2120x over previous
"""Euler-characteristic-curve kernel for Trainium2 (Bass/Tile), v2.

Per (batch, channel) group the reference computes
    cover(t_k) = #{n : birth_n < t_k <= death_n},  t_k = k/255 (f32)
and the output is cover_pd0 - cover_pd1.

Identity: [b < t][d >= t] = [b < t] - [max(b,d) < t], so everything
reduces to cumulative counts C(t_k) = #{v : v < t_k} of value streams.

Exact bin index per value: q = round(v*255) - [v < t_c] (int16), with
t_c = f32(c) * f32(1/255) matching the reference grid bit-exactly.

Counting scheme: with q = 16*qh + ql,
    C(16K+L) = Cc(K) + sum_p [qh_p == K][ql_p < L],   Cc = prefix(hist(qh))
Per 128-point pass the PE contracts
    A[p, .] = one-hot(qh)      (is_equal vs immediate, DVE 4x mode)
    B[p, .] = thermometer(ql)  (is_lt vs immediate, DVE 4x; col 0 = ones)
so PSUM accumulates, per stream, M[K,L] = joint prefix counts and
M[K,0] = hist(qh); the thermometer makes per-row scans unnecessary.
Rows/cols are interleaved (DG*bin + slot) so each pass's operand AP is a
single uniform-stride free dim.  Birth values accumulate into M0, max
values into M1; og = M0[d0]-M1[d0]-M0[d1]+M1[d1] (all four signs) falls
out of [+sel|-sel] selection matmuls at postproc, then one tiny
triangular matmul gives Cc and a broadcast-add finishes C.

Postprocessing of set s is emitted after compute of set s+1 so the
in-order ACT/DVE/PE streams never stall waiting on PSUM stops.

Sharding: data-parallel over batch, 4 batches per core x 8 cores.
"""

import os
import sys

for _p in ("/opt/trn_rl_repo", os.path.expanduser("~/.axon_site/_ro/trn_rl_repo")):
    if os.path.isdir(_p) and _p not in sys.path:
        sys.path.insert(0, _p)

import numpy as np

import concourse.bass as bass
import concourse.bacc as bacc
import concourse.mybir as mybir
from concourse.tile import TileContext
from concourse.bass_utils import run_bass_kernel_spmd

NCORES = 8
B, C, N = 32, 3, 8192
TT = 256
NG = (B // NCORES) * C        # 12 groups (b,c pairs) per diagram per core
R = float(np.float32(1.0) / np.float32(255.0))
SIZES = [4, 4, 4]             # groups per set (sum = NG, each <= 4)

F32 = mybir.dt.float32
BF16 = mybir.dt.bfloat16
I16 = mybir.dt.int16
OP = mybir.AluOpType
AF = mybir.ActivationFunctionType
P23 = 8388608.0               # 2^23


def build_nc():
    nc = bacc.Bacc("TRN2", target_bir_lowering=False, debug=False)
    pds = [
        nc.dram_tensor(f"pd{d}", [NG, N, 2], F32, kind="ExternalInput")
        for d in range(2)
    ]
    tri_d = nc.dram_tensor("tri", [16, 16], F32, kind="ExternalInput")
    sel_d = nc.dram_tensor("sel", [128, 256], F32, kind="ExternalInput")
    out_d = nc.dram_tensor("out", [NG, TT], F32, kind="ExternalOutput")

    with TileContext(nc) as tc:
        with (
            tc.tile_pool(name="consts", bufs=1) as cpool,
            tc.tile_pool(name="src", bufs=2) as spool,
            tc.tile_pool(name="prep", bufs=2) as tpool,
            tc.tile_pool(name="oh", bufs=2) as ohpool,
            tc.tile_pool(name="mm", bufs=2, space="PSUM") as ppool,
            tc.tile_pool(name="pcc", bufs=2, space="PSUM") as ccpool,
            tc.tile_pool(name="post", bufs=2) as qpool,
        ):
            tri = cpool.tile([16, 16], F32)
            sel = cpool.tile([128, 256], F32)

            goffs = np.cumsum([0] + SIZES[:-1]).tolist()

            def compute(sd):
                """DMA + prep + bin ops + main matmuls for set sd.
                Returns state needed by post(sd)."""
                G = SIZES[sd]
                goff = goffs[sd]
                W = 256 * G
                DG = 2 * G

                src = spool.tile([128, 2, G, 128], F32, tag="src", name="src")
                for d in range(2):
                    nc.sync.dma_start(
                        src[:, d, :, :],
                        pds[d]
                        .ap()[goff : goff + G, :, :]
                        .rearrange("g (p x) two -> p g (x two)", p=128),
                    )
                if sd == 0:
                    nc.sync.dma_start(tri[:, :], tri_d.ap())
                    nc.sync.dma_start(sel[:, :], sel_d.ap())

                flat = src[:, :, :, :].rearrange("p d g x -> p (d g x)")
                pairs = src[:, :, :, :].rearrange(
                    "p d g (i two) -> p (d g i) two", two=2
                )
                # deaths <- max(birth, death) in place
                nc.vector.tensor_tensor(
                    pairs[:, :, 1:2], pairs[:, :, 0:1], pairs[:, :, 1:2],
                    OP.max,
                )

                # exact bin index: c = round(v*255) via +2^23 trick; true
                # index is c - [v < t_c] with t_c = fl(c * R)
                tmb = tpool.tile([128, W], F32, tag="tmb", name="tmb")
                cf = tpool.tile([128, W], F32, tag="cf", name="cf")
                tlo = tpool.tile([128, W], F32, tag="tlo", name="tlo")
                ltf = tpool.tile([128, W], F32, tag="ltf", name="ltf")
                q = tpool.tile([128, W], I16, tag="q", name="q")
                nc.scalar.activation(
                    tmb[:, :], flat[:, :], AF.Copy, bias=P23, scale=255.0
                )
                nc.scalar.activation(cf[:, :], tmb[:, :], AF.Copy, bias=-P23)
                nc.scalar.mul(tlo[:, :], cf[:, :], float(R))
                nc.vector.tensor_tensor(
                    ltf[:, :], flat[:, :], tlo[:, :], OP.is_lt
                )
                nc.vector.tensor_tensor(q[:, :], cf[:, :], ltf[:, :], OP.subtract)
                qh = tpool.tile([128, W], I16, tag="qh", name="qh")
                ql = tpool.tile([128, W], I16, tag="ql", name="ql")
                nc.vector.tensor_scalar(
                    qh[:, :], q[:, :], 4, None, OP.logical_shift_right
                )
                nc.vector.tensor_scalar(
                    ql[:, :], q[:, :], 15, None, OP.bitwise_and
                )

                # bin tensors, [p, bin, d, g, i, v]; all DVE 4x ops
                A = ohpool.tile([128, 16, 2, G, 64, 2], BF16, tag="A", name="A")
                Bt = ohpool.tile([128, 16, 2, G, 64, 2], BF16, tag="B", name="B")
                Af = A[:, :, :, :, :, :].rearrange("p h d g i v -> p (h d g i v)")
                Bf = Bt[:, :, :, :, :, :].rearrange("p h d g i v -> p (h d g i v)")
                if sd < 2:
                    # ones col (L=0); buffers rotate with bufs=2 so later
                    # sets reuse the already-initialised region
                    nc.vector.memset(Bf[:, 0:W], 1.0)
                for h in range(16):
                    nc.vector.tensor_scalar(
                        Af[:, W * h : W * (h + 1)], qh[:, :],
                        h, None, OP.is_equal,
                    )
                for L in range(1, 16):
                    nc.vector.tensor_scalar(
                        Bf[:, W * L : W * (L + 1)], ql[:, :],
                        L, None, OP.is_lt,
                    )

                # PE: per (v, i) pass contract 128 points; interleaved
                # rows DG*h + slot, cols DG*L + slot (slot = G*d + g)
                M = [ppool.tile([DG * 16, DG * 16], F32, tag=f"M{v}",
                                name=f"M{v}")
                     for v in range(2)]

                def mk_ap(t, i, v):
                    ap = t[:, :, :, :, i, v]
                    # single free dim: addr = 128*(DG*bin + G*d + g)
                    return bass.AP(
                        ap.tensor, ap.offset,
                        [ap.ap[0], [ap.ap[3][0], DG * 16]],
                    )

                for v in range(2):
                    for i in range(64):
                        nc.tensor.matmul(
                            M[v][:, :],
                            mk_ap(A, i, v),
                            mk_ap(Bt, i, v),
                            start=(i == 0),
                            stop=(i == 63),
                        )
                return (sd, G, goff, DG, M)

            def post(state):
                sd, G, goff, DG, M = state
                # only one PSUM read per instruction: copy M0/M1 to SBUF
                net0 = qpool.tile([DG * 16, DG * 16], F32, tag="net0",
                                  name="net0")
                net1 = qpool.tile([DG * 16, DG * 16], F32, tag="net1",
                                  name="net1")
                nc.scalar.copy(net0[:, :], M[0][:, :])
                nc.scalar.copy(net1[:, :], M[1][:, :])
                # og_g = +M0[d0] - M1[d0] - M0[d1] + M1[d1] via selection
                # matmuls; sel[p, 16s+K] = [p == DG*K + s] (negated at +128)
                ogp = ccpool.tile([16, G, 16], F32, tag="ogp", name="ogp")

                def blkcols(net, s):
                    ap = net[:, :]
                    return bass.AP(
                        ap.tensor, ap.offset + s * ap.ap[1][0],
                        [ap.ap[0], [DG * ap.ap[1][0], 16]],
                    )

                for g in range(G):
                    s0, s1 = g, G + g
                    for k, (nt, s, neg) in enumerate(
                        ((net0, s0, 0), (net1, s0, 1),
                         (net0, s1, 1), (net1, s1, 0))
                    ):
                        c0 = 128 * neg + 16 * s
                        nc.tensor.matmul(
                            ogp[:, g, :],
                            sel[:, c0 : c0 + 16],
                            blkcols(nt, s),
                            start=(k == 0), stop=(k == 3),
                        )
                og = qpool.tile([16, G, 16], F32, tag="og", name="og")
                nc.scalar.copy(og[:, :, :], ogp[:, :, :])
                # coarse prefix: Cc[K, g] = sum_{K'<K} hist[K', g]
                ccp = ccpool.tile([16, G], F32, tag="ccp", name="ccp")
                nc.tensor.matmul(
                    ccp[:, :], tri[:, :], og[:, :, 0], start=True, stop=True
                )
                fin = qpool.tile([16, G, 16], F32, tag="fin", name="fin")
                nc.scalar.copy(fin[:, :, 0], ccp[:, :])
                for g in range(G):
                    nc.vector.tensor_scalar(
                        fin[:, g, 1:16], og[:, g, 1:16],
                        fin[:, g, 0:1], None, OP.add,
                    )
                nc.sync.dma_start(
                    out_d.ap()[goff : goff + G, :].rearrange(
                        "g (K L) -> K g L", K=16
                    ),
                    fin[:, :, :],
                )

            # defer each set's postproc past the next set's compute so
            # in-order engine streams don't stall on PSUM stops
            pending = []
            for sd in range(len(SIZES)):
                pending.append(compute(sd))
                if len(pending) > 1:
                    post(pending.pop(0))
            while pending:
                post(pending.pop(0))
    nc.compile()
    return nc


_NC = None


def _get_nc():
    global _NC
    if _NC is None:
        _NC = build_nc()
    return _NC


def make_in_maps(pd0, pd1):
    pd0 = np.ascontiguousarray(np.asarray(pd0, dtype=np.float32))
    pd1 = np.ascontiguousarray(np.asarray(pd1, dtype=np.float32))
    tri = (np.arange(16)[:, None] < np.arange(16)[None, :]).astype(np.float32)
    # sel[8K + s, 16s + K] = +1 (cols 0..127), -1 at cols 128..255
    sel = np.zeros((128, 256), dtype=np.float32)
    for s in range(8):
        for K in range(16):
            sel[8 * K + s, 16 * s + K] = 1.0
            sel[8 * K + s, 128 + 16 * s + K] = -1.0
    bs = B // NCORES
    in_maps = []
    for c in range(NCORES):
        in_maps.append(
            {
                "pd0": np.ascontiguousarray(
                    pd0[bs * c : bs * (c + 1)].reshape(NG, N, 2)
                ),
                "pd1": np.ascontiguousarray(
                    pd1[bs * c : bs * (c + 1)].reshape(NG, N, 2)
                ),
                "tri": tri,
                "sel": sel,
            }
        )
    return in_maps


def kernel(pd0, pd1, trace=False):
    nc = _get_nc()
    in_maps = make_in_maps(pd0, pd1)
    res = run_bass_kernel_spmd(nc, in_maps, list(range(NCORES)), trace=trace)
    bs = B // NCORES
    out = np.concatenate(
        [res.results[c]["out"].reshape(bs, C, TT) for c in range(NCORES)], axis=0
    )
    if trace:
        return out.astype(np.float32), res
    return out.astype(np.float32)
